# revision 23
# baseline (speedup 1.0000x reference)
"""Trainium2 Bass kernel for nn_JSONTreeLSTM (K=8192, L=128, D=64) on 8 NeuronCores.

Strategy
--------
Data-parallel over K: each core gets 1024 rows of x. The NumberEmbedder is
rank-1 (emb = x*w + b), so the LSTM input projection and the gate bias fold
into two extra contraction rows of the recurrent matmul: the stationary
operand is [W_pair.T; u_pair; b_pair] with shape [66, 128]; the moving
operand is the [66, W] tile holding h (rows 0:64), the DMA'd x_t row
(row 64) and a constant ones row (row 65). All matmul operands are bf16
(1 cycle/col on TensorE vs 4 for fp32 — the v1 kernel was 98.8%
TensorE-bound because of fp32 matmuls).

Gate packing (sigma(2g) = (tanh(g)+1)/2 keeps each bank single-function;
the layout keeps every DVE op on uniform input partition bases and puts
tg/pa in the ACT window between the two sigmoids, off the critical path):
  bank0 = [i; 2g]  -> sigma(i) at parts 0:64,  sigma(2g) at parts 64:128
  bank1 = [f; o]   -> sigma(f) at parts 0:64,  sigma(o) at parts 64:128

Per step t, per k-group (3 groups of ~341, software-pipelined with a
third-step stagger between the groups):
  PG0 = S0.T @ rhs ; PG1 = S1.T @ rhs   (TensorE, PSUM fp32)
  SB0 = sigmoid(PG0) ; SB1 = sigmoid(PG1)  (ACT, one op per bank)
  tg  = 2*sigma2g - 1                  (DVE tensor_scalar = tanh(g))
  pa  = sigmai * tg                    (DVE tensor_tensor)
  pb  = sigmaf * C                     (DVE tensor_tensor)
  C   = pa + pb                        (DVE tensor_add, in place)
  Th  = tanh(C)                        (ACT)
  h2  = sigmao * Th                    (DVE -> rows 0:64 of next rhs)

The object-level reduction needs only per-core partial sums sum_k(h_L) and
sum_k(sigmoid(f)*c); f is computed with host-composed weights
(W_fh@W_aout). The tiny [1,64] object LSTM tail runs on host in float64.
"""

import os
import sys

import numpy as np

sys.path.insert(0, "/opt/trn_rl_repo")

import concourse.bass as bass
import concourse.mybir as mybir
import concourse.tile as tile
from concourse import bacc, bass_utils

K, L, D = 8192, 128, 64
NCORES = 8
KSH = K // NCORES      # 1024 rows per core
NG = 3                 # k-groups per core
WS = [344, 344, 336]   # per-group batch widths (sum = KSH)
GS = [0, 344, 688]     # per-group column offsets
W = 344                # max group width (tile sizing)
BK = 512               # PSUM bank stride (fp32 cols); bank1 starts here
NSLOT = 4              # rhs rotation depth
F32 = mybir.dt.float32
BF16 = mybir.dt.bfloat16
AF = mybir.ActivationFunctionType
ALU = mybir.AluOpType

_CACHE: dict = {}


def _sigmoid(z):
    return 1.0 / (1.0 + np.exp(-z))


def _prep_weights(inp):
    """Compose the device weight tiles (float64 math, cast on return)."""
    f = {k: np.asarray(v, np.float64) for k, v in inp.items()}
    W_ih_h = f["W_ih"][:, :D]                       # [256, 64]
    u = W_ih_h @ f["W_num"][:, 0]                   # [256]
    bias = f["b_ih"] + f["b_hh"] + W_ih_h @ f["b_num"]
    W_hh = f["W_hh"]                                # [256, 64]; rows i,f,g,o
    # S0: [66, 128] = [[Wi.T | 2*Wg.T]; [u_i | 2*u_g]; [b_i | 2*b_g]]
    S0 = np.zeros((66, 128))
    S0[0:64, 0:64] = W_hh[0:64].T
    S0[0:64, 64:128] = 2.0 * W_hh[128:192].T
    S0[64, 0:64] = u[0:64]
    S0[64, 64:128] = 2.0 * u[128:192]
    S0[65, 0:64] = bias[0:64]
    S0[65, 64:128] = 2.0 * bias[128:192]
    # S1: [66, 128] = [[Wf.T | Wo.T]; [u_f | u_o]; [b_f | b_o]]
    S1 = np.zeros((66, 128))
    S1[0:64, 0:64] = W_hh[64:128].T
    S1[0:64, 64:128] = W_hh[192:256].T
    S1[64, 0:64] = u[64:128]
    S1[64, 64:128] = u[192:256]
    S1[65, 0:64] = bias[64:128]
    S1[65, 64:128] = bias[192:256]
    # composed f-gate for the object reduction
    Wcomb = f["W_fh"] @ f["W_aout"]
    LF = Wcomb.T                                    # [64, 64]
    BIASF = (f["W_fh"] @ f["b_aout"] + f["b_fh"]).reshape(64, 1)
    return (S0.astype(np.float32), S1.astype(np.float32),
            LF.astype(np.float32), np.ascontiguousarray(BIASF, np.float32))


def _build_nc(n_steps=L):
    nc = bacc.Bacc("TRN2")
    xT_d = nc.dram_tensor("xT", [L, KSH], BF16, kind="ExternalInput")
    S0_d = nc.dram_tensor("S0", [66, 128], F32, kind="ExternalInput")
    S1_d = nc.dram_tensor("S1", [66, 128], F32, kind="ExternalInput")
    LF_d = nc.dram_tensor("LF", [64, 64], F32, kind="ExternalInput")
    BIASF_d = nc.dram_tensor("BIASF", [64, 1], F32, kind="ExternalInput")
    out_d = nc.dram_tensor("out", [64, 6], F32, kind="ExternalOutput")

    with tile.TileContext(nc) as tc:
        with (
            tc.tile_pool(name="singles", bufs=1) as singles,
            tc.tile_pool(name="sab", bufs=3) as sab_pool,
            tc.tile_pool(name="pp", bufs=3) as p_pool,
            tc.tile_pool(name="th", bufs=3) as th_pool,
            tc.tile_pool(name="fin", bufs=1) as fin_pool,
            tc.tile_pool(name="psum", bufs=1, space="PSUM") as psum_pool,
        ):
            s0f = singles.tile([66, 128], F32, tag="s0f")
            s1f = singles.tile([66, 128], F32, tag="s1f")
            s0 = singles.tile([66, 128], BF16, tag="s0")
            s1 = singles.tile([66, 128], BF16, tag="s1")
            lff = singles.tile([64, 64], F32, tag="lff")
            lf = singles.tile([64, 64], BF16, tag="lf")
            biasf = singles.tile([64, 1], F32, tag="biasf")
            nc.sync.dma_start(s0f, S0_d[:, :])
            nc.sync.dma_start(s1f, S1_d[:, :])
            nc.sync.dma_start(lff, LF_d[:, :])
            nc.sync.dma_start(biasf, BIASF_d[:, :])
            nc.vector.tensor_copy(s0, s0f)
            nc.vector.tensor_copy(s1, s1f)
            nc.vector.tensor_copy(lf, lff)

            # rhs rotation slots: [66, KSH] bf16; rows 0:64 = h (per-group
            # column blocks), row 64 = x_t (one DMA covers both groups),
            # row 65 = const ones (bias row)
            RH = [singles.tile([66, KSH], BF16, tag=f"rh{i}", name=f"rh{i}")
                  for i in range(NSLOT)]
            # C state lives at partitions 0:64, matching sigma(f)'s base so
            # every DVE op sees uniform input partition bases
            CS = [singles.tile([128, W], BF16, tag=f"cs{g}", name=f"cs{g}")
                  for g in range(NG)]
            for i in range(NSLOT):
                # rows 64:66 <- 1.0 (start-64 keeps the partition-quad rule;
                # the per-step x DMA overwrites row 64, row 65 stays ones)
                nc.vector.memset(RH[i][64:66, :], 1.0)
            nc.vector.memset(RH[0][0:64, :], 0.0)
            for g in range(NG):
                nc.vector.memset(CS[g][0:64, :], 0.0)
            # x row for step 0 into slot 0
            nc.sync.dma_start(RH[0][64:65, :], xT_d[0:1, :])

            # Three-group software pipeline. Per group one [128, 1024]
            # PSUM tile spans 2 banks: bank0 matmul writes cols 0:wg,
            # bank1 matmul writes cols 512:512+wg (bank-aligned starts);
            # one sigmoid covers the whole 0:512+wg span (the gap columns
            # are unused garbage).
            SAB = [None] * NG
            PA = [None] * NG

            def block1(g, t):
                rh_t = RH[t % NSLOT]
                gs, wg = GS[g], WS[g]
                pg = psum_pool.tile([128, 2 * BK], F32, tag=f"pg{g}")
                nc.tensor.matmul(pg[:, 0:wg], s0, rh_t[:, gs:gs + wg],
                                 start=True, stop=True)
                nc.tensor.matmul(pg[:, BK:BK + wg], s1, rh_t[:, gs:gs + wg],
                                 start=True, stop=True)
                sab = sab_pool.tile([128, 2 * BK], BF16, tag=f"sab{g}")
                nc.scalar.activation(sab[:, 0:BK + wg], pg[:, 0:BK + wg],
                                     AF.Sigmoid)
                # tg = tanh(g) = 2*sigma(2g) - 1   [in parts 64:128 -> out 0:64]
                tg = p_pool.tile([64, W], BF16, tag=f"tg{g}", name=f"tg{g}")
                nc.vector.tensor_scalar(tg[:, 0:wg], sab[64:128, 0:wg],
                                        2.0, 1.0, ALU.mult, ALU.subtract)
                # pa = sigmai * tg   [parts 0:64]
                pa = p_pool.tile([64, W], BF16, tag=f"pa{g}", name=f"pa{g}")
                nc.vector.tensor_mul(pa[:, 0:wg], sab[0:64, 0:wg],
                                     tg[:, 0:wg])
                SAB[g], PA[g] = sab, pa

            def block2(g, t):
                rh_n = RH[(t + 1) % NSLOT]
                gs, wg = GS[g], WS[g]
                sab, pa = SAB[g], PA[g]
                # pb = sigmaf * C_old   [parts 0:64] — on GpSimd, off the
                # saturated DVE ring (arrives no later than the DVE would
                # reach it behind tg/pa)
                pb = p_pool.tile([64, W], BF16, tag=f"pb{g}", name=f"pb{g}")
                nc.gpsimd.tensor_mul(pb[:, 0:wg], sab[0:64, BK:BK + wg],
                                     CS[g][0:64, 0:wg])
                # C = pa + pb (in place at parts 0:64)
                nc.vector.tensor_add(CS[g][0:64, 0:wg], pa[:, 0:wg],
                                     pb[:, 0:wg])
                # tanh(C) lands at parts 64:128 so h2's inputs share a base
                th = th_pool.tile([128, W], BF16, tag=f"th{g}")
                nc.scalar.activation(th[64:128, 0:wg], CS[g][0:64, 0:wg],
                                     AF.Tanh)
                # h2 = sigmao * tanh(C)  [ins at parts 64:128]
                nc.vector.tensor_mul(rh_n[0:64, gs:gs + wg],
                                     sab[64:128, BK:BK + wg],
                                     th[64:128, 0:wg])

            for t in range(n_steps):
                if t + 1 < n_steps:
                    # prefetch x_{t+1} into the next slot's row 64
                    nc.sync.dma_start(RH[(t + 1) % NSLOT][64:65, :],
                                      xT_d[t + 1:t + 2, :])
                block1(0, t)
                if t > 0:
                    block2(2, t - 1)
                block1(1, t)
                block2(0, t)
                block1(2, t)
                block2(1, t)
            block2(2, n_steps - 1)

            # ---- final per-core partials ----
            rh_f = RH[n_steps % NSLOT]
            for g in range(NG):
                gs, wg = GS[g], WS[g]
                pf = psum_pool.tile([64, W], F32, tag=f"pg{g}")
                nc.tensor.matmul(pf[:, 0:wg], lf, rh_f[0:64, gs:gs + wg],
                                 start=True, stop=True)
                sf = fin_pool.tile([64, W], BF16, tag=f"sf{g}")
                nc.scalar.activation(sf[:, 0:wg], pf[:, 0:wg], AF.Sigmoid,
                                     bias=biasf)
                scr = fin_pool.tile([64, W], BF16, tag=f"scr{g}")
                fcs = fin_pool.tile([64, 1], F32, tag=f"fcs{g}")
                nc.vector.scalar_tensor_tensor(
                    scr[:, 0:wg], sf[:, 0:wg], 1.0, CS[g][0:64, 0:wg],
                    ALU.mult, ALU.mult, accum_out=fcs)
                hs = fin_pool.tile([64, 1], F32, tag=f"hs{g}")
                nc.vector.tensor_reduce(hs, rh_f[0:64, gs:gs + wg],
                                        mybir.AxisListType.X, ALU.add)
                nc.sync.dma_start(out_d[:, g:g + 1], hs)
                nc.sync.dma_start(out_d[:, 3 + g:4 + g], fcs)

    nc.finalize()
    return nc


def _get_nc(n_steps=L):
    key = ("nc", n_steps)
    if key not in _CACHE:
        _CACHE[key] = _build_nc(n_steps)
    return _CACHE[key]


def _run_device(xT16, S0, S1, LF, BIASF, trace=False, n_steps=L):
    nc = _get_nc(n_steps)
    in_maps = []
    for c in range(NCORES):
        xs = np.ascontiguousarray(xT16[:, c * KSH:(c + 1) * KSH])
        in_maps.append({"xT": xs, "S0": S0, "S1": S1,
                        "LF": LF, "BIASF": BIASF})
    import time
    t0 = time.time()
    res = bass_utils.run_bass_kernel_spmd(
        nc, in_maps, core_ids=list(range(NCORES)), trace=trace)
    _run_device.last_wall_s = time.time() - t0
    return res


def kernel(**inputs):
    import ml_dtypes
    inp = {k: np.asarray(v) for k, v in inputs.items()}
    S0, S1, LF, BIASF = _prep_weights(inp)
    xT16 = np.asarray(inp["x"], np.float32).T.astype(ml_dtypes.bfloat16)
    trace = bool(int(os.environ.get("LSTM_TRACE", "0")))
    res = _run_device(xT16, S0, S1, LF, BIASF, trace=trace)
    kernel._last_exec_ns = res.exec_time_ns
    hsum = np.zeros(64, np.float64)
    fcs = np.zeros(64, np.float64)
    for r in res.results:
        o = np.asarray(r["out"], np.float64)
        hsum += o[:, 0] + o[:, 1] + o[:, 2]
        fcs += o[:, 3] + o[:, 4] + o[:, 5]
    # ---- host: object-level TreeLSTM tail (tiny) ----
    f = {k: np.asarray(v, np.float64) for k, v in inp.items()}
    hs_bar = hsum @ f["W_aout"].T + K * f["b_aout"]
    iou = hs_bar @ f["W_iouh"].T + f["b_iouh"]
    i, o_, u = iou[0:64], iou[64:128], iou[128:192]
    c_obj = _sigmoid(i) * np.tanh(u) + fcs
    h_obj = _sigmoid(o_) * np.tanh(c_obj)
    h_hat = h_obj @ f["W_oout"].T + f["b_oout"]
    return np.concatenate([h_hat, c_obj])[None].astype(np.float32)


kernel._last_exec_ns = None
